# revision 2
# baseline (speedup 1.0000x reference)
"""Trainium2 Bass kernel for nn_DagnabbitAutoEncoder — v2.

Key ideas vs the AllGather-per-level baseline:
- Re-layer the DAG by TRUE dependency depth (longest parent chain): this
  input has depth ~19 instead of 64 topological levels, so only ~19 serial
  phases, each with ~2100 nodes/core -> large matmuls, amortized LDWEIGHTS.
- The 8 NeuronCores share physical HBM ("Shared" scratchpad). Each core
  scatter-writes its computed rows once into a shared embeddings buffer at
  host-assigned addresses; peers gather exactly the rows they need. The only
  cross-core sync is a tiny 4-byte AllGather barrier per layer whose output
  lands in the shared buffer so that the next layer's gathers (whose declared
  window covers the barrier rows) pick up a proper RAW dependency.
- Multi-column indirect DMA: one instruction gathers/scatters all chunks of
  a layer (~994ns fixed + 0.34ns/row), instead of one instruction per
  128 rows.
- Per layer: gather parents (node-major), PE-transpose to feature-major,
  grouped 2-layer MLP per encoder type (f16 weights, f32 PSUM accum, exact
  Gelu on ScalarE between layers), PE-transpose back, scatter + barrier.
  Parents in layers <= k-2 are gathered one layer early (overlapping
  compute), only parents in the immediately previous layer wait on its
  barrier.
"""

import numpy as np

D = 128
R = 1024
T = 4
OUT_SLOTS = 256
START = T
L = 64
M = 2048
N = R + L * M
NC = 8
_P = 128

SLOTS_BASE = R                    # slot-embedding rows
SCRATCH = R + OUT_SLOTS           # 8 per-core scratch rows
SB0 = SCRATCH + NC                # first slab row


def _preprocess(par, typ):
    """Host-side: true-depth layers, per-core slot assignment, index tables."""
    # true DAG depth per node (roots = 0)
    layer = np.zeros(N, np.int32)
    for l in range(L):
        lo = R + l * M
        pl = layer[par[lo - R:lo - R + M]]      # (M, 2)
        layer[lo:lo + M] = pl.max(1) + 1
    ND = int(layer.max())

    e_idx = np.where(typ >= START, T, typ).astype(np.int64)   # (N-R,)
    slot_id = np.clip(typ - START, 0, OUT_SLOTS - 1).astype(np.int64)

    buf_row = np.full(N, -1, np.int64)
    buf_row[:R] = np.arange(R)

    layers = []
    slab = SB0
    # assignment: node -> (k, core, slot); buf_row[v] = slab_k + c*C_k + slot
    for k in range(1, ND + 1):
        in_k = np.nonzero(layer[R:] == k)[0]            # node idx - R
        groups = []
        s = 0
        percore = [[] for _ in range(NC)]
        for e in range(T + 1):
            b = in_k[e_idx[in_k] == e]
            if len(b) == 0:
                continue
            # class-sort so old/fresh parents form contiguous chunk runs:
            # rank 0 = both parents old, 1 = p0 old/p1 fresh,
            # 2 = both fresh, 3 = p0 fresh/p1 old
            f0 = layer[par[b, 0]] == k - 1
            is_out = typ[b] >= START
            f1 = (~is_out) & (layer[par[b, 1]] == k - 1)
            rank = np.where(~f0 & ~f1, 0,
                            np.where(~f0 & f1, 1, np.where(f0 & f1, 2, 3)))
            b = b[np.argsort(rank, kind="stable")]
            g = (len(b) + NC - 1) // NC
            groups.append((e, s, s + g))
            for c in range(NC):
                mine = b[c::NC]
                percore[c].append((s, mine))
            s += g
        cols = s
        nch = (cols + _P - 1) // _P
        C = nch * _P
        for c in range(NC):
            for s0, mine in percore[c]:
                buf_row[R + mine] = slab + c * C + s0 + np.arange(len(mine))
        layers.append(dict(k=k, cols=cols, nch=nch, C=C, groups=groups,
                           percore=percore, slab=slab))
        slab += NC * C

    BARBASE = slab
    C_max = max(lv["C"] for lv in layers)
    ROWS_TOTAL = BARBASE + ND + 8 + C_max
    assert (buf_row[R:] >= 0).all()

    # index tables: per layer columns [p0 old | p0 fresh | p1 old | p1 fresh |
    # scatter]; simpler: p0 (nch), p1 (nch), scatter (nch) -- with an
    # old/fresh chunk split recorded for the gather emission.
    nch_tot = sum(lv["nch"] for lv in layers)
    QT = 3 * nch_tot
    idx_tab = np.zeros((NC, _P, QT), np.int32)
    out_rows = 0
    qb = 0
    for ki, lv in enumerate(layers):
        nch, C, slabk = lv["nch"], lv["C"], lv["slab"]
        prev_k = lv["k"] - 1
        # per-core per-slot node ids (or -1 padding)
        p0g = np.zeros((NC, C), np.int64)
        p1g = np.zeros((NC, C), np.int64)
        scat = np.zeros((NC, C), np.int64)
        for c in range(NC):
            nodes = np.full(C, -1, np.int64)
            for s0, mine in lv["percore"][c]:
                nodes[s0:s0 + len(mine)] = mine
            real = nodes >= 0
            vr = nodes[real]
            p0 = par[vr, 0]
            p1 = par[vr, 1]
            is_out = typ[vr] >= START
            p0r = buf_row[p0]
            p1r = np.where(is_out, SLOTS_BASE + slot_id[vr], buf_row[p1])
            a0 = np.zeros(C, np.int64)
            a1 = np.zeros(C, np.int64)
            sc = np.full(C, SCRATCH + c, np.int64)
            a0[real] = p0r
            a1[real] = p1r
            sc[real] = slabk + c * C + np.nonzero(real)[0]
            p0g[c], p1g[c], scat[c] = a0, a1, sc
        idx_tab[:, :, qb:qb + nch] = p0g.reshape(NC, nch, _P).transpose(0, 2, 1)
        idx_tab[:, :, qb + nch:qb + 2 * nch] = p1g.reshape(NC, nch, _P).transpose(0, 2, 1)
        idx_tab[:, :, qb + 2 * nch:qb + 3 * nch] = scat.reshape(NC, nch, _P).transpose(0, 2, 1)
        lv["qb"] = qb
        lv["outbase"] = out_rows
        out_rows += C
        qb += 3 * nch
    return dict(layers=layers, idx_tab=idx_tab, QT=QT, ND=ND,
                BARBASE=BARBASE, ROWS_TOTAL=ROWS_TOTAL, out_rows=out_rows,
                buf_row=buf_row, layer=layer)


def _build_program(pp):
    import concourse.bass as bass
    import concourse.bacc as bacc
    import concourse.mybir as mybir
    import concourse.tile as tile
    from concourse.masks import make_identity
    from concourse.tile_rust import add_dep_helper
    from concourse.expressions import s_logical_and

    if not hasattr(bass.log, "warn_once_per_message"):
        bass.log.warn_once_per_message = bass.log.warn_once_per_call_stack

    f32 = mybir.dt.float32
    f16 = mybir.dt.float16
    i32 = mybir.dt.int32
    nc = bacc.Bacc("TRN2", target_bir_lowering=False, debug=False,
                   num_devices=NC)

    layers, BARBASE, ROWS_TOTAL = pp["layers"], pp["BARBASE"], pp["ROWS_TOTAL"]
    ND = pp["ND"]
    nch_max = max(lv["nch"] for lv in layers)

    root_p = nc.declare_dram_parameter("root", [R, D], f16, isOutput=False)
    w1_p = nc.declare_dram_parameter("w1", [T + 1, 2 * D, 2 * D], f16, isOutput=False)
    b1_p = nc.declare_dram_parameter("b1", [T + 1, 2 * D], f32, isOutput=False)
    w2_p = nc.declare_dram_parameter("w2", [T + 1, 2 * D, D], f16, isOutput=False)
    b2_p = nc.declare_dram_parameter("b2", [T + 1, D], f32, isOutput=False)
    slots_p = nc.declare_dram_parameter("slots", [OUT_SLOTS, D], f16, isOutput=False)
    idx_p = nc.declare_dram_parameter("idx", [_P, pp["QT"]], i32, isOutput=False)
    cid_p = nc.declare_dram_parameter("cid", [1, 1], mybir.dt.uint32, isOutput=False)
    out_p = nc.declare_dram_parameter("out", [pp["out_rows"], D], f16, isOutput=True)
    diag_rows = NC * layers[0]["C"]
    diag_p = nc.declare_dram_parameter("diag", [diag_rows, D], f16, isOutput=True)

    buf = nc.dram_tensor("buf", [ROWS_TOTAL, D], f16, addr_space="Shared")
    stage = nc.dram_tensor("stage", [pp["out_rows"], D], f16)

    gelu = mybir.ActivationFunctionType.Gelu
    BLK = 512

    with tile.TileContext(nc) as tc:
        with tc.tile_pool(name="const", bufs=1) as const, \
             tc.tile_pool(name="gat", bufs=2) as gat, \
             tc.tile_pool(name="xt", bufs=2) as xtp, \
             tc.tile_pool(name="ht", bufs=2) as htp, \
             tc.tile_pool(name="et", bufs=2) as etp, \
             tc.tile_pool(name="esb", bufs=2) as esbp, \
             tc.tile_pool(name="tps", bufs=2, space="PSUM") as tps, \
             tc.tile_pool(name="hps", bufs=1, space="PSUM") as hps, \
             tc.tile_pool(name="eps", bufs=2, space="PSUM") as eps:

            ident = const.tile([_P, _P], f16)
            make_identity(nc, ident[:])

            idx_t = const.tile([_P, pp["QT"]], i32)
            nc.sync.dma_start(idx_t[:], idx_p[:])
            w1_r = const.tile([_P, (T + 1) * 4, _P], f16)   # [(t,kc,mc)]
            nc.sync.dma_start(
                w1_r[:].rearrange("p (t k m) o -> p t k m o", t=T + 1, k=2),
                w1_p[:].rearrange("t (k p) (m o) -> p t k m o", p=_P, o=_P))
            w2_r = const.tile([_P, (T + 1) * 2, _P], f16)   # [(t,kc)]
            nc.sync.dma_start(
                w2_r[:].rearrange("p (t k) o -> p t k o", t=T + 1),
                w2_p[:].rearrange("t (k p) o -> p t k o", p=_P))
            b1_t = const.tile([_P, (T + 1) * 2], f32)       # [(t,mc)]
            nc.sync.dma_start(
                b1_t[:].rearrange("p (t m) -> p t m", t=T + 1),
                b1_p[:].rearrange("t (m p) -> p t m", p=_P))
            b2_t = const.tile([_P, T + 1], f32)
            nc.sync.dma_start(b2_t[:], b2_p[:].rearrange("t p -> p t"))

            # preload roots + slot embeddings into shared buf (all cores write
            # identical bytes -> benign)
            pre1 = nc.sync.dma_start(
                buf[0:R].rearrange("(a p) d -> p a d", p=_P),
                root_p[:].rearrange("(a p) d -> p a d", p=_P))
            pre2 = nc.sync.dma_start(
                buf[SLOTS_BASE:SLOTS_BASE + OUT_SLOTS].rearrange(
                    "(a p) d -> p a d", p=_P),
                slots_p[:].rearrange("(a p) d -> p a d", p=_P))

            # PE warmup
            warm = tps.tile([_P, _P], f16, tag="tp")
            nc.tensor.transpose(out=warm[:], in_=ident[:], identity=ident[:])

            # core id from a per-core input (partition_id() is unreliable
            # under the bass2jax/axon execution path)
            cid_t = const.tile([1, 1], mybir.dt.uint32)
            nc.sync.dma_start(cid_t[:], cid_p[:])
            cid_reg = nc.gpsimd.alloc_register("cid_reg")
            nc.gpsimd.reg_load(cid_reg, cid_t[0:1, 0:1])
            pid = nc.gpsimd.snap(cid_reg, donate=True, min_val=0, max_val=NC - 1)

            gtiles = {}

            def emit_gathers(ki, which):
                """which: 'old' gathers (window excludes prev layer's barrier)
                or 'fresh' (window includes it)."""
                lv = layers[ki]
                nch, qb = lv["nch"], lv["qb"]
                k = lv["k"]
                if ki not in gtiles:
                    ga = gat.tile([_P, nch_max, _P], f16, tag="ga", name=f"ga{ki}")
                    gb = gat.tile([_P, nch_max, _P], f16, tag="gb", name=f"gb{ki}")
                    gtiles[ki] = (ga, gb)
                ga, gb = gtiles[ki]
                # windows at slab boundaries: "fresh" needs the previous
                # layer's AllGathered slab, "old" only slabs before it
                win_old = layers[ki - 1]["slab"] if ki >= 1 else SB0
                win_fresh = lv["slab"]
                for side, base, tile_ in ((0, qb, ga), (1, qb + nch, gb)):
                    o, f = lv["oldfresh"][side]
                    if which == "old":
                        for q in range(0, o):
                            nc.gpsimd.indirect_dma_start(
                                out=tile_[:, q, :], out_offset=None,
                                in_=buf[0:win_old],
                                in_offset=bass.IndirectOffsetOnAxis(
                                    ap=idx_t[0:_P, base + q:base + q + 1],
                                    axis=0))
                    if which == "fresh":
                        for q in range(o, f):
                            nc.gpsimd.indirect_dma_start(
                                out=tile_[:, q, :], out_offset=None,
                                in_=buf[0:win_fresh],
                                in_offset=bass.IndirectOffsetOnAxis(
                                    ap=idx_t[0:_P, base + q:base + q + 1],
                                    axis=0))

            # old/fresh split: which leading chunks of each side are fully
            # "old" (no parent row in the previous layer's slab)
            for ki, lv in enumerate(layers):
                prev = layers[ki - 1] if ki > 0 else None
                lo = prev["slab"] if prev else ROWS_TOTAL
                hi = prev["slab"] + NC * prev["C"] if prev else ROWS_TOTAL
                of = []
                for side in range(2):
                    base = lv["qb"] + side * lv["nch"]
                    tabs = pp["idx_tab"][0][:, base:base + lv["nch"]]
                    # chunk is old iff no index in [lo, hi) on ANY core
                    old = lv["nch"]
                    for q in range(lv["nch"]):
                        col = pp["idx_tab"][:, :, base + q]
                        if ((col >= lo) & (col < hi)).any():
                            old = q
                            break
                    of.append((old, lv["nch"]))
                lv["oldfresh"] = of

            emit_gathers(0, "old")

            for ki, lv in enumerate(layers):
                k, nch, cols, C = lv["k"], lv["nch"], lv["cols"], lv["C"]
                qb, slabk = lv["qb"], lv["slab"]

                emit_gathers(ki, "fresh")
                ga, gb = gtiles.pop(ki)

                x0T = xtp.tile([_P, nch_max * _P], f16, tag="x0", name=f"x0{ki}")
                x1T = xtp.tile([_P, nch_max * _P], f16, tag="x1", name=f"x1{ki}")
                for q in range(nch):
                    for src, dst in ((ga, x0T), (gb, x1T)):
                        tp = tps.tile([_P, _P], f16, tag="tp", name=f"t{ki}_{q}_{0 if src is ga else 1}")
                        nc.tensor.transpose(out=tp[:], in_=src[:, q, :],
                                            identity=ident[:])
                        nc.vector.tensor_copy(dst[:, q * _P:(q + 1) * _P], tp[:])

                e_sb = esbp.tile([_P, nch_max, _P], f16, tag="esb", name=f"esb{ki}")

                nblk = (cols + BLK - 1) // BLK
                for b in range(nblk):
                    b0 = b * BLK
                    b1 = min(cols, b0 + BLK)
                    bgroups = [(e, max(s, b0), min(t, b1))
                               for (e, s, t) in lv["groups"]
                               if t > b0 and s < b1]
                    h_ps = [hps.tile([_P, BLK], f32, tag=f"h{mc}",
                                     name=f"h{mc}_{ki}_{b}") for mc in range(2)]
                    hT = [htp.tile([_P, BLK], f16, tag=f"hT{mc}",
                                   name=f"hT{mc}_{ki}_{b}") for mc in range(2)]
                    e_ps = eps.tile([_P, BLK], f32, tag="e", name=f"e_{ki}_{b}")
                    eT = etp.tile([_P, BLK], f16, tag="eT", name=f"eT{ki}_{b}")
                    for (e, s, t) in bgroups:
                        for kc, srct in ((0, x0T), (1, x1T)):
                            for mc in range(2):
                                nc.tensor.matmul(
                                    h_ps[mc][:, s - b0:t - b0],
                                    lhsT=w1_r[:, (e * 2 + kc) * 2 + mc, :],
                                    rhs=srct[:, s:t],
                                    start=(kc == 0), stop=(kc == 1))
                    for mc in range(2):
                        for (e, s, t) in bgroups:
                            nc.scalar.activation(
                                hT[mc][:, s - b0:t - b0],
                                h_ps[mc][:, s - b0:t - b0], gelu,
                                bias=b1_t[:, e * 2 + mc:e * 2 + mc + 1],
                                scale=1.0)
                    for (e, s, t) in bgroups:
                        for kc in range(2):
                            nc.tensor.matmul(
                                e_ps[:, s - b0:t - b0],
                                lhsT=w2_r[:, e * 2 + kc, :],
                                rhs=hT[kc][:, s - b0:t - b0],
                                start=(kc == 0), stop=(kc == 1))
                    for (e, s, t) in bgroups:
                        nc.vector.tensor_scalar(
                            out=eT[:, s - b0:t - b0], in0=e_ps[:, s - b0:t - b0],
                            scalar1=b2_t[:, e:e + 1], scalar2=None,
                            op0=mybir.AluOpType.add)
                    # out-transpose chunks fully inside this block
                    q0 = b0 // _P
                    q1 = (b1 + _P - 1) // _P
                    for q in range(q0, q1):
                        tp2 = tps.tile([_P, _P], f16, tag="tp", name=f"o{ki}_{q}")
                        nc.tensor.transpose(out=tp2[:],
                                            in_=eT[:, q * _P - b0:(q + 1) * _P - b0],
                                            identity=ident[:])
                        nc.vector.tensor_copy(e_sb[:, q, :], tp2[:])

                # own rows -> per-core output buffer (plain, per-core tensor)
                ob = lv["outbase"]
                nc.sync.dma_start(
                    out_p[ob:ob + C].rearrange("(q p) d -> p q d", p=_P),
                    e_sb[:, 0:nch, :])
                # exchange: own rows -> Local stage (static per-core tensor),
                # then AllGather the layer slice into the shared slab
                # (rank-major concat == buf_row layout). Cross-chip data can
                # only move via the fabric collective.
                nc.sync.dma_start(
                    stage[ob:ob + C].rearrange("(q p) d -> p q d", p=_P),
                    e_sb[:, 0:nch, :])
                if ki + 1 < ND:
                    lastag = nc.gpsimd.collective_compute(
                        "AllGather", mybir.AluOpType.bypass,
                        replica_groups=[list(range(NC))],
                        ins=[stage[ob:ob + C]],
                        outs=[buf[slabk:slabk + NC * C]])
                    emit_gathers(ki + 1, "old")

            dump = nc.sync.dma_start(
                diag_p[:].rearrange("(a p) d -> p a d", p=_P),
                buf[SB0:SB0 + diag_rows].rearrange("(a p) d -> p a d", p=_P))
            add_dep_helper(dump.ins, lastag.ins, reason="diag after last AG")

    nc.compile()
    return nc


def kernel(root_node_embeddings, W1, b1, W2, b2, out_slot_emb,
           node_inputs_indices, node_types, _trace=False):
    from concourse.bass_utils import run_bass_kernel_spmd

    par = np.asarray(node_inputs_indices).astype(np.int64).reshape(N - R, 2)
    typ = np.asarray(node_types).astype(np.int64).reshape(N - R)

    pp = _preprocess(par, typ)
    nc = _build_program(pp)

    common = {
        "root": np.ascontiguousarray(np.asarray(root_node_embeddings, dtype=np.float16)),
        "w1": np.ascontiguousarray(np.asarray(W1, dtype=np.float16)),
        "b1": np.ascontiguousarray(np.asarray(b1, dtype=np.float32)),
        "w2": np.ascontiguousarray(np.asarray(W2, dtype=np.float16)),
        "b2": np.ascontiguousarray(np.asarray(b2, dtype=np.float32)),
        "slots": np.ascontiguousarray(np.asarray(out_slot_emb, dtype=np.float16)),
    }
    in_maps = [dict(common, idx=np.ascontiguousarray(pp["idx_tab"][c]),
                    cid=np.array([[c]], dtype=np.uint32))
               for c in range(NC)]

    res = run_bass_kernel_spmd(nc, in_maps, list(range(NC)), trace=_trace)

    try:
        np.save("/tmp/v2diag.npy",
                np.stack([res.results[c]["diag"] for c in range(NC)]))
    except Exception:
        pass

    out_full = np.empty((N, D), dtype=np.float32)
    out_full[:R] = np.asarray(root_node_embeddings, dtype=np.float32)
    for lv in pp["layers"]:
        ob = lv["outbase"]
        for c in range(NC):
            rows = res.results[c]["out"][ob:ob + lv["C"]]
            for s0, mine in lv["percore"][c]:
                out_full[R + mine] = rows[s0:s0 + len(mine)].astype(np.float32)

    if _trace:
        return out_full, res
    return out_full


# revision 3
# speedup vs baseline: 1.0078x; 1.0078x over previous
"""Trainium2 Bass kernel for nn_DagnabbitAutoEncoder — v2.

Key ideas vs the AllGather-per-level baseline:
- Re-layer the DAG by TRUE dependency depth (longest parent chain): this
  input has depth ~19 instead of 64 topological levels, so only ~19 serial
  phases, each with ~2100 nodes/core -> large matmuls, amortized LDWEIGHTS.
- The 8 NeuronCores share physical HBM ("Shared" scratchpad). Each core
  scatter-writes its computed rows once into a shared embeddings buffer at
  host-assigned addresses; peers gather exactly the rows they need. The only
  cross-core sync is a tiny 4-byte AllGather barrier per layer whose output
  lands in the shared buffer so that the next layer's gathers (whose declared
  window covers the barrier rows) pick up a proper RAW dependency.
- Multi-column indirect DMA: one instruction gathers/scatters all chunks of
  a layer (~994ns fixed + 0.34ns/row), instead of one instruction per
  128 rows.
- Per layer: gather parents (node-major), PE-transpose to feature-major,
  grouped 2-layer MLP per encoder type (f16 weights, f32 PSUM accum, exact
  Gelu on ScalarE between layers), PE-transpose back, scatter + barrier.
  Parents in layers <= k-2 are gathered one layer early (overlapping
  compute), only parents in the immediately previous layer wait on its
  barrier.
"""

import numpy as np

D = 128
R = 1024
T = 4
OUT_SLOTS = 256
START = T
L = 64
M = 2048
N = R + L * M
NC = 8
_P = 128

SLOTS_BASE = R                    # slot-embedding rows
SCRATCH = R + OUT_SLOTS           # 8 per-core scratch rows
SB0 = SCRATCH + NC                # first slab row


def _preprocess(par, typ):
    """Host-side: true-depth layers, per-core slot assignment, index tables."""
    # true DAG depth per node (roots = 0)
    layer = np.zeros(N, np.int32)
    for l in range(L):
        lo = R + l * M
        pl = layer[par[lo - R:lo - R + M]]      # (M, 2)
        layer[lo:lo + M] = pl.max(1) + 1
    ND = int(layer.max())

    e_idx = np.where(typ >= START, T, typ).astype(np.int64)   # (N-R,)
    slot_id = np.clip(typ - START, 0, OUT_SLOTS - 1).astype(np.int64)

    buf_row = np.full(N, -1, np.int64)
    buf_row[:R] = np.arange(R)

    layers = []
    slab = SB0
    # assignment: node -> (k, core, slot); buf_row[v] = slab_k + c*C_k + slot
    for k in range(1, ND + 1):
        in_k = np.nonzero(layer[R:] == k)[0]            # node idx - R
        groups = []
        s = 0
        percore = [[] for _ in range(NC)]
        for e in range(T + 1):
            b = in_k[e_idx[in_k] == e]
            if len(b) == 0:
                continue
            # class-sort so old/fresh parents form contiguous chunk runs:
            # rank 0 = both parents old, 1 = p0 old/p1 fresh,
            # 2 = both fresh, 3 = p0 fresh/p1 old
            f0 = layer[par[b, 0]] == k - 1
            is_out = typ[b] >= START
            f1 = (~is_out) & (layer[par[b, 1]] == k - 1)
            rank = np.where(~f0 & ~f1, 0,
                            np.where(~f0 & f1, 1, np.where(f0 & f1, 2, 3)))
            b = b[np.argsort(rank, kind="stable")]
            g = (len(b) + NC - 1) // NC
            groups.append((e, s, s + g))
            for c in range(NC):
                mine = b[c::NC]
                percore[c].append((s, mine))
            s += g
        cols = s
        nch = (cols + _P - 1) // _P
        C = nch * _P
        for c in range(NC):
            for s0, mine in percore[c]:
                buf_row[R + mine] = slab + c * C + s0 + np.arange(len(mine))
        layers.append(dict(k=k, cols=cols, nch=nch, C=C, groups=groups,
                           percore=percore, slab=slab))
        slab += NC * C

    BARBASE = slab
    C_max = max(lv["C"] for lv in layers)
    ROWS_TOTAL = BARBASE + ND + 8 + C_max
    assert (buf_row[R:] >= 0).all()

    # index tables: per layer columns [p0 old | p0 fresh | p1 old | p1 fresh |
    # scatter]; simpler: p0 (nch), p1 (nch), scatter (nch) -- with an
    # old/fresh chunk split recorded for the gather emission.
    nch_tot = sum(lv["nch"] for lv in layers)
    QT = 3 * nch_tot
    idx_tab = np.zeros((NC, _P, QT), np.int32)
    out_rows = 0
    qb = 0
    for ki, lv in enumerate(layers):
        nch, C, slabk = lv["nch"], lv["C"], lv["slab"]
        prev_k = lv["k"] - 1
        # per-core per-slot node ids (or -1 padding)
        p0g = np.zeros((NC, C), np.int64)
        p1g = np.zeros((NC, C), np.int64)
        scat = np.zeros((NC, C), np.int64)
        for c in range(NC):
            nodes = np.full(C, -1, np.int64)
            for s0, mine in lv["percore"][c]:
                nodes[s0:s0 + len(mine)] = mine
            real = nodes >= 0
            vr = nodes[real]
            p0 = par[vr, 0]
            p1 = par[vr, 1]
            is_out = typ[vr] >= START
            p0r = buf_row[p0]
            p1r = np.where(is_out, SLOTS_BASE + slot_id[vr], buf_row[p1])
            a0 = np.zeros(C, np.int64)
            a1 = np.zeros(C, np.int64)
            sc = np.full(C, SCRATCH + c, np.int64)
            a0[real] = p0r
            a1[real] = p1r
            sc[real] = slabk + c * C + np.nonzero(real)[0]
            p0g[c], p1g[c], scat[c] = a0, a1, sc
        idx_tab[:, :, qb:qb + nch] = p0g.reshape(NC, nch, _P).transpose(0, 2, 1)
        idx_tab[:, :, qb + nch:qb + 2 * nch] = p1g.reshape(NC, nch, _P).transpose(0, 2, 1)
        idx_tab[:, :, qb + 2 * nch:qb + 3 * nch] = scat.reshape(NC, nch, _P).transpose(0, 2, 1)
        lv["qb"] = qb
        lv["outbase"] = out_rows
        out_rows += C
        qb += 3 * nch
    return dict(layers=layers, idx_tab=idx_tab, QT=QT, ND=ND,
                BARBASE=BARBASE, ROWS_TOTAL=ROWS_TOTAL, out_rows=out_rows,
                buf_row=buf_row, layer=layer)


def _build_program(pp):
    import concourse.bass as bass
    import concourse.bacc as bacc
    import concourse.mybir as mybir
    import concourse.tile as tile
    from concourse.masks import make_identity
    from concourse.tile_rust import add_dep_helper
    from concourse.expressions import s_logical_and

    if not hasattr(bass.log, "warn_once_per_message"):
        bass.log.warn_once_per_message = bass.log.warn_once_per_call_stack

    f32 = mybir.dt.float32
    f16 = mybir.dt.float16
    i32 = mybir.dt.int32
    nc = bacc.Bacc("TRN2", target_bir_lowering=False, debug=False,
                   num_devices=NC)

    layers, BARBASE, ROWS_TOTAL = pp["layers"], pp["BARBASE"], pp["ROWS_TOTAL"]
    ND = pp["ND"]
    nch_max = max(lv["nch"] for lv in layers)

    root_p = nc.declare_dram_parameter("root", [R, D], f16, isOutput=False)
    w1_p = nc.declare_dram_parameter("w1", [T + 1, 2 * D, 2 * D], f16, isOutput=False)
    b1_p = nc.declare_dram_parameter("b1", [T + 1, 2 * D], f32, isOutput=False)
    w2_p = nc.declare_dram_parameter("w2", [T + 1, 2 * D, D], f16, isOutput=False)
    b2_p = nc.declare_dram_parameter("b2", [T + 1, D], f32, isOutput=False)
    slots_p = nc.declare_dram_parameter("slots", [OUT_SLOTS, D], f16, isOutput=False)
    idx_p = nc.declare_dram_parameter("idx", [_P, pp["QT"]], i32, isOutput=False)
    cid_p = nc.declare_dram_parameter("cid", [1, 1], mybir.dt.uint32, isOutput=False)
    out_p = nc.declare_dram_parameter("out", [pp["out_rows"], D], f16, isOutput=True)
    diag_rows = NC * layers[0]["C"]
    diag_p = nc.declare_dram_parameter("diag", [diag_rows, D], f16, isOutput=True)

    buf = nc.dram_tensor("buf", [ROWS_TOTAL, D], f16, addr_space="Shared")
    stage = nc.dram_tensor("stage", [pp["out_rows"], D], f16)

    gelu = mybir.ActivationFunctionType.Gelu
    BLK = 512

    with tile.TileContext(nc) as tc:
        with tc.tile_pool(name="const", bufs=1) as const, \
             tc.tile_pool(name="gat", bufs=2) as gat, \
             tc.tile_pool(name="xt", bufs=2) as xtp, \
             tc.tile_pool(name="ht", bufs=2) as htp, \
             tc.tile_pool(name="et", bufs=2) as etp, \
             tc.tile_pool(name="esb", bufs=2) as esbp, \
             tc.tile_pool(name="tps", bufs=2, space="PSUM") as tps, \
             tc.tile_pool(name="hps", bufs=1, space="PSUM") as hps, \
             tc.tile_pool(name="eps", bufs=2, space="PSUM") as eps:

            ident = const.tile([_P, _P], f16)
            make_identity(nc, ident[:])

            idx_t = const.tile([_P, pp["QT"]], i32)
            nc.sync.dma_start(idx_t[:], idx_p[:])
            w1_r = const.tile([_P, (T + 1) * 4, _P], f16)   # [(t,kc,mc)]
            nc.sync.dma_start(
                w1_r[:].rearrange("p (t k m) o -> p t k m o", t=T + 1, k=2),
                w1_p[:].rearrange("t (k p) (m o) -> p t k m o", p=_P, o=_P))
            w2_r = const.tile([_P, (T + 1) * 2, _P], f16)   # [(t,kc)]
            nc.sync.dma_start(
                w2_r[:].rearrange("p (t k) o -> p t k o", t=T + 1),
                w2_p[:].rearrange("t (k p) o -> p t k o", p=_P))
            b1_t = const.tile([_P, (T + 1) * 2], f32)       # [(t,mc)]
            nc.sync.dma_start(
                b1_t[:].rearrange("p (t m) -> p t m", t=T + 1),
                b1_p[:].rearrange("t (m p) -> p t m", p=_P))
            b2_t = const.tile([_P, T + 1], f32)
            nc.sync.dma_start(b2_t[:], b2_p[:].rearrange("t p -> p t"))

            # preload roots + slot embeddings into shared buf (all cores write
            # identical bytes -> benign)
            pre1 = nc.sync.dma_start(
                buf[0:R].rearrange("(a p) d -> p a d", p=_P),
                root_p[:].rearrange("(a p) d -> p a d", p=_P))
            pre2 = nc.sync.dma_start(
                buf[SLOTS_BASE:SLOTS_BASE + OUT_SLOTS].rearrange(
                    "(a p) d -> p a d", p=_P),
                slots_p[:].rearrange("(a p) d -> p a d", p=_P))

            # PE warmup
            warm = tps.tile([_P, _P], f16, tag="tp")
            nc.tensor.transpose(out=warm[:], in_=ident[:], identity=ident[:])

            # core id from a per-core input (partition_id() is unreliable
            # under the bass2jax/axon execution path)
            cid_t = const.tile([1, 1], mybir.dt.uint32)
            nc.sync.dma_start(cid_t[:], cid_p[:])
            cid_reg = nc.gpsimd.alloc_register("cid_reg")
            nc.gpsimd.reg_load(cid_reg, cid_t[0:1, 0:1])
            pid = nc.gpsimd.snap(cid_reg, donate=True, min_val=0, max_val=NC - 1)

            gtiles = {}

            def emit_gathers(ki, which):
                """which: 'old' gathers (window excludes prev layer's barrier)
                or 'fresh' (window includes it)."""
                lv = layers[ki]
                nch, qb = lv["nch"], lv["qb"]
                k = lv["k"]
                if ki not in gtiles:
                    ga = gat.tile([_P, nch_max, _P], f16, tag="ga", name=f"ga{ki}")
                    gb = gat.tile([_P, nch_max, _P], f16, tag="gb", name=f"gb{ki}")
                    gtiles[ki] = (ga, gb)
                ga, gb = gtiles[ki]
                # windows at slab boundaries: "fresh" needs the previous
                # layer's AllGathered slab, "old" only slabs before it
                win_old = layers[ki - 1]["slab"] if ki >= 1 else SB0
                win_fresh = lv["slab"]
                want_old = which == "old"
                win = win_old if want_old else win_fresh
                for side, base, tile_ in ((0, qb, ga), (1, qb + nch, gb)):
                    oldq = lv["oldfresh"][side]
                    for q in range(nch):
                        if oldq[q] != want_old:
                            continue
                        nc.gpsimd.indirect_dma_start(
                            out=tile_[:, q, :], out_offset=None,
                            in_=buf[0:win],
                            in_offset=bass.IndirectOffsetOnAxis(
                                ap=idx_t[0:_P, base + q:base + q + 1],
                                axis=0))

            # old/fresh split: per-chunk classification (class-sorted slots
            # make old chunks contiguous per type group, at arbitrary
            # positions). A chunk is "old" iff no index on ANY core points
            # into the previous layer's slab.
            for ki, lv in enumerate(layers):
                prev = layers[ki - 1] if ki > 0 else None
                lo = prev["slab"] if prev else ROWS_TOTAL
                hi = prev["slab"] + NC * prev["C"] if prev else ROWS_TOTAL
                of = []
                for side in range(2):
                    base = lv["qb"] + side * lv["nch"]
                    oldq = []
                    for q in range(lv["nch"]):
                        col = pp["idx_tab"][:, :, base + q]
                        oldq.append(not ((col >= lo) & (col < hi)).any())
                    of.append(oldq)
                lv["oldfresh"] = of

            emit_gathers(0, "old")

            for ki, lv in enumerate(layers):
                k, nch, cols, C = lv["k"], lv["nch"], lv["cols"], lv["C"]
                qb, slabk = lv["qb"], lv["slab"]

                emit_gathers(ki, "fresh")
                ga, gb = gtiles.pop(ki)

                x0T = xtp.tile([_P, nch_max * _P], f16, tag="x0", name=f"x0{ki}")
                x1T = xtp.tile([_P, nch_max * _P], f16, tag="x1", name=f"x1{ki}")
                for q in range(nch):
                    for src, dst in ((ga, x0T), (gb, x1T)):
                        tp = tps.tile([_P, _P], f16, tag="tp", name=f"t{ki}_{q}_{0 if src is ga else 1}")
                        nc.tensor.transpose(out=tp[:], in_=src[:, q, :],
                                            identity=ident[:])
                        nc.vector.tensor_copy(dst[:, q * _P:(q + 1) * _P], tp[:])

                e_sb = esbp.tile([_P, nch_max, _P], f16, tag="esb", name=f"esb{ki}")

                nblk = (cols + BLK - 1) // BLK
                for b in range(nblk):
                    b0 = b * BLK
                    b1 = min(cols, b0 + BLK)
                    bgroups = [(e, max(s, b0), min(t, b1))
                               for (e, s, t) in lv["groups"]
                               if t > b0 and s < b1]
                    h_ps = [hps.tile([_P, BLK], f32, tag=f"h{mc}",
                                     name=f"h{mc}_{ki}_{b}") for mc in range(2)]
                    hT = [htp.tile([_P, BLK], f16, tag=f"hT{mc}",
                                   name=f"hT{mc}_{ki}_{b}") for mc in range(2)]
                    e_ps = eps.tile([_P, BLK], f32, tag="e", name=f"e_{ki}_{b}")
                    eT = etp.tile([_P, BLK], f16, tag="eT", name=f"eT{ki}_{b}")
                    for (e, s, t) in bgroups:
                        for kc, srct in ((0, x0T), (1, x1T)):
                            for mc in range(2):
                                nc.tensor.matmul(
                                    h_ps[mc][:, s - b0:t - b0],
                                    lhsT=w1_r[:, (e * 2 + kc) * 2 + mc, :],
                                    rhs=srct[:, s:t],
                                    start=(kc == 0), stop=(kc == 1))
                    for mc in range(2):
                        for (e, s, t) in bgroups:
                            nc.scalar.activation(
                                hT[mc][:, s - b0:t - b0],
                                h_ps[mc][:, s - b0:t - b0], gelu,
                                bias=b1_t[:, e * 2 + mc:e * 2 + mc + 1],
                                scale=1.0)
                    for (e, s, t) in bgroups:
                        for kc in range(2):
                            nc.tensor.matmul(
                                e_ps[:, s - b0:t - b0],
                                lhsT=w2_r[:, e * 2 + kc, :],
                                rhs=hT[kc][:, s - b0:t - b0],
                                start=(kc == 0), stop=(kc == 1))
                    for (e, s, t) in bgroups:
                        nc.vector.tensor_scalar(
                            out=eT[:, s - b0:t - b0], in0=e_ps[:, s - b0:t - b0],
                            scalar1=b2_t[:, e:e + 1], scalar2=None,
                            op0=mybir.AluOpType.add)
                    # out-transpose chunks fully inside this block
                    q0 = b0 // _P
                    q1 = (b1 + _P - 1) // _P
                    for q in range(q0, q1):
                        tp2 = tps.tile([_P, _P], f16, tag="tp", name=f"o{ki}_{q}")
                        nc.tensor.transpose(out=tp2[:],
                                            in_=eT[:, q * _P - b0:(q + 1) * _P - b0],
                                            identity=ident[:])
                        nc.vector.tensor_copy(e_sb[:, q, :], tp2[:])

                # own rows -> per-core output buffer (plain, per-core tensor)
                ob = lv["outbase"]
                nc.sync.dma_start(
                    out_p[ob:ob + C].rearrange("(q p) d -> p q d", p=_P),
                    e_sb[:, 0:nch, :])
                # exchange: own rows -> Local stage (static per-core tensor),
                # then AllGather the layer slice into the shared slab
                # (rank-major concat == buf_row layout). Cross-chip data can
                # only move via the fabric collective.
                nc.sync.dma_start(
                    stage[ob:ob + C].rearrange("(q p) d -> p q d", p=_P),
                    e_sb[:, 0:nch, :])
                if ki + 1 < ND:
                    lastag = nc.gpsimd.collective_compute(
                        "AllGather", mybir.AluOpType.bypass,
                        replica_groups=[list(range(NC))],
                        ins=[stage[ob:ob + C]],
                        outs=[buf[slabk:slabk + NC * C]])
                    emit_gathers(ki + 1, "old")

            dump = nc.sync.dma_start(
                diag_p[:].rearrange("(a p) d -> p a d", p=_P),
                buf[SB0:SB0 + diag_rows].rearrange("(a p) d -> p a d", p=_P))
            add_dep_helper(dump.ins, lastag.ins, reason="diag after last AG")

    nc.compile()
    return nc


def kernel(root_node_embeddings, W1, b1, W2, b2, out_slot_emb,
           node_inputs_indices, node_types, _trace=False):
    from concourse.bass_utils import run_bass_kernel_spmd

    par = np.asarray(node_inputs_indices).astype(np.int64).reshape(N - R, 2)
    typ = np.asarray(node_types).astype(np.int64).reshape(N - R)

    pp = _preprocess(par, typ)
    nc = _build_program(pp)

    common = {
        "root": np.ascontiguousarray(np.asarray(root_node_embeddings, dtype=np.float16)),
        "w1": np.ascontiguousarray(np.asarray(W1, dtype=np.float16)),
        "b1": np.ascontiguousarray(np.asarray(b1, dtype=np.float32)),
        "w2": np.ascontiguousarray(np.asarray(W2, dtype=np.float16)),
        "b2": np.ascontiguousarray(np.asarray(b2, dtype=np.float32)),
        "slots": np.ascontiguousarray(np.asarray(out_slot_emb, dtype=np.float16)),
    }
    in_maps = [dict(common, idx=np.ascontiguousarray(pp["idx_tab"][c]),
                    cid=np.array([[c]], dtype=np.uint32))
               for c in range(NC)]

    res = run_bass_kernel_spmd(nc, in_maps, list(range(NC)), trace=_trace)

    try:
        np.save("/tmp/v2diag.npy",
                np.stack([res.results[c]["diag"] for c in range(NC)]))
    except Exception:
        pass

    out_full = np.empty((N, D), dtype=np.float32)
    out_full[:R] = np.asarray(root_node_embeddings, dtype=np.float32)
    for lv in pp["layers"]:
        ob = lv["outbase"]
        for c in range(NC):
            rows = res.results[c]["out"][ob:ob + lv["C"]]
            for s0, mine in lv["percore"][c]:
                out_full[R + mine] = rows[s0:s0 + len(mine)].astype(np.float32)

    if _trace:
        return out_full, res
    return out_full
